# revision 13
# baseline (speedup 1.0000x reference)
"""DenseEdgeConv (B=4, N=2048, D=64, K=16, C=128) on 8 trn2 cores.

Sharding: data-parallel over (batch, half-of-N): core c handles batch c//2,
query rows (c%2)*1024 ... +1024. Each core receives its batch's full point
set (rotated so its query rows are columns 0..1023 — this keeps the SPMD
program identical across cores) and produces [448, 1024] (channels, rows).

Per-core pipeline, per 128-row block:
  1. PE: neg-distance matmul  2*q.p - |p|^2  (k=6) -> PSUM [128, 2048]
  2. ACT evict to SBUF; DVE adds -1e30 on the self-diagonal
  3. DVE max/max_index/match_replace x2 -> 16 nearest neighbor indices
  4. idx relayout via DRAM bounce -> gpsimd ap_gather of x_j [64, 2048 edges]
  5. PE MLP chain per 512-edge subtile (biases folded via ones-row, k=65):
       h1 = relu(W1ac^T x_i + W1bc^T x_j + b1)   (ACT relu on eviction)
       h2 = relu(W2h^T h1 + W2x^T x_i + b2)
       h3 = W3h2^T h2 + W3h1^T h1 + W3x^T x_i + b3 (stays in PSUM)
  6. DVE reduce_max over K -> h1max/h2max/h3max; DMA out
Output channel layout: [h3 (0:128) | h2 (128:256) | h1 (256:384) | x (384:448)].
"""

import numpy as np

import concourse.bass as bass
import concourse.mybir as mybir
from concourse.tile import TileContext

B, N, D, K, C = 4, 2048, 64, 16, 128
NCORES = 8
Q = 1024          # query rows per core
NBLK = Q // 128   # row blocks per core
NSUB = 4          # 512-edge subtiles per block
SUB = 512
NEG_BIG = -1.0e30
REPL_VAL = -1.0e38

F32 = mybir.dt.float32
F32R = mybir.dt.float32r
U16 = mybir.dt.uint16
I16 = mybir.dt.int16

USE_FP32R = False  # flip to use full-rate fp32 matmuls (verify numerics on HW)


_MAX_WAITS = 1       # walrus limit for compute/DMA instructions
_MAX_WAITS_DMA = 1   # DMA instructions only have one wait slot
_NOP_WAITS = 1       # universal: one wait slot per instruction
_DMA_OPS = {"DMACopy", "DMATranspose", "TensorLoad", "TensorSave", "DMATrigger"}


def _split_excess_waits(m):
    """Move sync waits beyond the per-opcode limit onto NoOps inserted just
    before the over-limit instruction (same engine, so stream order is
    preserved)."""
    n_new = 0
    for fn in m["functions"]:
        for bb in fn["blocks"]:
            out = []
            for inst in bb["instructions"]:
                si = inst.get("sync_info")
                waits = (si or {}).get("on_wait") or []
                lim = _MAX_WAITS_DMA if inst["opcode"] in _DMA_OPS else _MAX_WAITS
                if len(waits) > lim:
                    extra = waits[: len(waits) - lim]
                    si["on_wait"] = waits[len(waits) - lim :]
                    while extra:
                        chunk, extra = extra[:_NOP_WAITS], extra[_NOP_WAITS:]
                        n_new += 1
                        out.append(
                            {
                                "debug": inst.get("debug", 0),
                                "engine": inst["engine"],
                                "ins": [],
                                "outs": [],
                                "name": f"{inst['name']}-wsplit{n_new}",
                                "opcode": "NoOp",
                                "sync_info": {"on_update": [], "on_wait": chunk},
                            }
                        )
                out.append(inst)
            bb["instructions"] = out
    return m


def _install_wait_split_hook():
    import json

    if getattr(bass.Bass, "_wait_split_patched", False):
        return
    orig = bass.Bass.to_json_bytes

    def to_json_bytes(self, *a, **kw):
        raw = orig(self, *a, **kw)
        m = json.loads(raw)
        m = _split_excess_waits(m)
        return json.dumps(m).encode()

    bass.Bass.to_json_bytes = to_json_bytes
    bass.Bass._wait_split_patched = True


def _install_tile_tail_workaround():
    """This neuronxcc build rejects the stock TileContext tail (Drain with
    sem waits / eq-waits -> 'Too many sync wait commands'). Use plain
    per-engine drains + sem-only barriers instead, the same pattern
    bass.Block.__exit__ uses with no_gpsimd_drain."""
    from concourse import tile as _tile

    if getattr(_tile.TileContext, "_tail_patched", False):
        return

    def _drain_and_barrier(self, tick_clock, wait_clock):
        nc = self.nc
        for eng_type, eng in nc.engines.items():
            d = mybir.InstDrain(
                name=nc.get_next_instruction_name(),
                ins=[],
                outs=[],
                bass_is_fusable=False,
            )
            d.engine = eng_type
            eng.add_instruction(d)
        nc.all_engine_barrier(sem_only=True)
        popped = nc._tile_sem_poison_stack.pop()
        assert popped is self._sem_poison
        nc.clear_and_free_semaphores(list(self.sems.allocated().values()))
        nc.all_engine_barrier(sem_only=True)

    _tile.TileContext._drain_and_barrier = _drain_and_barrier
    _tile.TileContext._tail_patched = True


def _mmcast(ap):
    return ap.bitcast(F32R) if USE_FP32R else ap


def build_nc():
    _install_tile_tail_workaround()
    _install_wait_split_hook()
    nc = bass.Bass("TRN2")

    xT = nc.dram_tensor("xT", [D, N], F32, kind="ExternalInput")
    posT = nc.dram_tensor("posT", [3, N], F32, kind="ExternalInput")
    W1d = nc.dram_tensor("W1", [3 * D, C], F32, kind="ExternalInput")
    W2d = nc.dram_tensor("W2", [D + C, C], F32, kind="ExternalInput")
    W3d = nc.dram_tensor("W3", [D + 2 * C, C], F32, kind="ExternalInput")
    b1d = nc.dram_tensor("b1", [1, C], F32, kind="ExternalInput")
    b2d = nc.dram_tensor("b2", [1, C], F32, kind="ExternalInput")
    b3d = nc.dram_tensor("b3", [1, C], F32, kind="ExternalInput")
    ndiagd = nc.dram_tensor("ndiag", [128, 128], F32, kind="ExternalInput")
    o = nc.dram_tensor("o", [3 * C + D, Q], F32, kind="ExternalOutput")

    with TileContext(nc) as tc:
        with (
            tc.tile_pool(name="setup", bufs=1) as setup,
            tc.tile_pool(name="wtmp", bufs=1) as wtmp,
            tc.tile_pool(name="negp", bufs=1, space="PSUM") as negpool,
            tc.tile_pool(name="hp", bufs=1, space="PSUM") as hpool,
            tc.tile_pool(name="sel", bufs=2) as sel,
            tc.tile_pool(name="mlp", bufs=2) as mlp,
            tc.tile_pool(name="dscr", bufs=2, space="DRAM") as dscr,
        ):
            # ---- setup ----
            xTo = setup.tile([D + 1, N], F32)         # x^T with a ones row
            nc.sync.dma_start(xTo[0:D, :], xT[:, :])
            nc.vector.memset(xTo[D : D + 1, :], 1.0)

            # gather source must span 128 partitions (indirect_copy); only
            # rows 0:64 are ever read back
            xg = setup.tile([128, N], F32)
            nc.sync.dma_start(xg[0:D, :], xT[:, :])
            nc.vector.memset(xg[D:128, :], 0.0)

            rhs6 = setup.tile([6, N], F32)            # [pos; pos^2]
            nc.sync.dma_start(rhs6[0:3, :], posT[:, :])
            sq = setup.tile([3, N], F32)
            nc.vector.tensor_tensor(
                out=sq[:, :], in0=rhs6[0:3, :], in1=rhs6[0:3, :],
                op=mybir.AluOpType.mult,
            )
            nc.sync.dma_start(rhs6[3:6, :], sq[:, :])
            lhsT6 = setup.tile([6, Q], F32)           # [2*q; -1]
            nc.vector.memset(lhsT6[:, :], -1.0)
            nc.vector.tensor_scalar_mul(lhsT6[0:3, :], rhs6[0:3, 0:Q], 2.0)

            ndiag = setup.tile([128, 128], F32)
            nc.sync.dma_start(ndiag[:, :], ndiagd[:, :])

            # ---- weights ----
            w1a = wtmp.tile([D, C], F32, tag="w1a")
            w1c = wtmp.tile([D, C], F32, tag="w1c")
            nc.sync.dma_start(w1a[:, :], W1d[0:D, :])
            nc.sync.dma_start(w1c[:, :], W1d[2 * D : 3 * D, :])

            L1a = setup.tile([D + 1, C], F32)   # W1a - W1c ; b1
            L1b = setup.tile([D, C], F32)       # W1b + W1c
            nc.vector.tensor_tensor(
                out=L1a[0:D, :], in0=w1a[:, :], in1=w1c[:, :],
                op=mybir.AluOpType.subtract,
            )
            nc.sync.dma_start(L1a[D : D + 1, :], b1d[:, :])
            nc.sync.dma_start(L1b[:, :], W1d[D : 2 * D, :])
            nc.vector.tensor_tensor(
                out=L1b[:, :], in0=L1b[:, :], in1=w1c[:, :],
                op=mybir.AluOpType.add,
            )

            L2h = setup.tile([C, C], F32)
            L2x = setup.tile([D + 1, C], F32)
            nc.sync.dma_start(L2h[:, :], W2d[0:C, :])
            nc.sync.dma_start(L2x[0:D, :], W2d[C : C + D, :])
            nc.sync.dma_start(L2x[D : D + 1, :], b2d[:, :])

            L3h2 = setup.tile([C, C], F32)
            L3h1 = setup.tile([C, C], F32)
            L3x = setup.tile([D + 1, C], F32)
            nc.sync.dma_start(L3h2[:, :], W3d[0:C, :])
            nc.sync.dma_start(L3h1[:, :], W3d[C : 2 * C, :])
            nc.sync.dma_start(L3x[0:D, :], W3d[2 * C : 2 * C + D, :])
            nc.sync.dma_start(L3x[D : D + 1, :], b3d[:, :])

            # x passthrough rows of the output
            nc.sync.dma_start(o[3 * C : 3 * C + D, :], xTo[0:D, 0:Q])

            for blk in range(NBLK):
                c0 = blk * 128  # this block's query rows / diag columns

                # -- distances --
                negp = negpool.tile([128, N], F32, tag="negp")
                for j4 in range(4):
                    nc.tensor.matmul(
                        negp[:, j4 * 512 : (j4 + 1) * 512],
                        _mmcast(lhsT6[:, c0 : c0 + 128]),
                        _mmcast(rhs6[:, j4 * 512 : (j4 + 1) * 512]),
                        start=True,
                        stop=True,
                    )

                neg_sb = sel.tile([128, N], F32, tag="neg")
                if c0 > 0:
                    nc.scalar.activation(
                        neg_sb[:, 0:c0], negp[:, 0:c0],
                        mybir.ActivationFunctionType.Copy,
                    )
                if c0 + 128 < N:
                    nc.scalar.activation(
                        neg_sb[:, c0 + 128 : N], negp[:, c0 + 128 : N],
                        mybir.ActivationFunctionType.Copy,
                    )
                nc.vector.tensor_tensor(
                    out=neg_sb[:, c0 : c0 + 128],
                    in0=negp[:, c0 : c0 + 128],
                    in1=ndiag[:, :],
                    op=mybir.AluOpType.add,
                )

                # -- top-16 selection --
                v8 = sel.tile([128, 8], F32, tag="v8")
                idx = sel.tile([128, K], U16, tag="idx")
                neg2 = sel.tile([128, N], F32, tag="neg2")
                nc.vector.max(v8[:, :], neg_sb[:, :])
                nc.vector.max_index(idx[:, 0:8], v8[:, :], neg_sb[:, :])
                nc.vector.match_replace(neg2[:, :], v8[:, :], neg_sb[:, :], REPL_VAL)
                v8b = sel.tile([128, 8], F32, tag="v8b")
                nc.vector.max(v8b[:, :], neg2[:, :])
                nc.vector.max_index(idx[:, 8:16], v8b[:, :], neg2[:, :])

                # -- idx relayout: [row, k] -> [k, row] in DRAM, read back
                #    replicated to all 8 gpsimd core groups --
                scr = dscr.tile([K, 128], U16)
                nc.sync.dma_start(scr[:, :].transpose([1, 0]), idx[:, :])
                idxw = mlp.tile([128, 128], U16, tag="idxw")
                nc.sync.dma_start(
                    idxw[:, :],
                    scr[:, :].unsqueeze(0).broadcast_to([8, K, 128]),
                )

                # -- gather x_j (edge t = r*16 + k), 512 idxs max per inst --
                xj = mlp.tile([128, 2048], F32, tag="xj")
                for s in range(NSUB):
                    nc.gpsimd.indirect_copy(
                        xj[:, s * SUB : (s + 1) * SUB],
                        xg[:, :],
                        idxw[:, s * (SUB // 16) : (s + 1) * (SUB // 16)],
                        i_know_ap_gather_is_preferred=True,
                    )

                h1max = mlp.tile([128, 128], F32, tag="h1max")
                h2max = mlp.tile([128, 128], F32, tag="h2max")
                h3max = mlp.tile([128, 128], F32, tag="h3max")

                for s in range(NSUB):
                    e0 = s * SUB
                    r0 = c0 + s * (SUB // K)
                    # x_i expanded: each query column repeated 16x
                    xi = (
                        _mmcast(xTo[:, r0 : r0 + SUB // K])
                        .unsqueeze(2)
                        .broadcast_to([D + 1, SUB // K, K])
                    )

                    h1p = hpool.tile([128, SUB], F32, tag="h1p")
                    nc.tensor.matmul(
                        h1p[:, :], _mmcast(L1a[:, :]), xi, start=True, stop=False
                    )
                    nc.tensor.matmul(
                        h1p[:, :], _mmcast(L1b[:, :]),
                        _mmcast(xj[0:D, e0 : e0 + SUB]),
                        start=False, stop=True,
                    )
                    h1sb = mlp.tile([128, SUB], F32, tag="h1sb")
                    nc.scalar.activation(
                        h1sb[:, :], h1p[:, :], mybir.ActivationFunctionType.Relu
                    )

                    h2p = hpool.tile([128, SUB], F32, tag="h2p")
                    nc.tensor.matmul(
                        h2p[:, :], _mmcast(L2h[:, :]), _mmcast(h1sb[:, :]),
                        start=True, stop=False,
                    )
                    nc.tensor.matmul(
                        h2p[:, :], _mmcast(L2x[:, :]), xi, start=False, stop=True
                    )
                    h2sb = mlp.tile([128, SUB], F32, tag="h2sb")
                    nc.scalar.activation(
                        h2sb[:, :], h2p[:, :], mybir.ActivationFunctionType.Relu
                    )

                    h3p = hpool.tile([128, SUB], F32, tag="h3p")
                    nc.tensor.matmul(
                        h3p[:, :], _mmcast(L3h2[:, :]), _mmcast(h2sb[:, :]),
                        start=True, stop=False,
                    )
                    nc.tensor.matmul(
                        h3p[:, :], _mmcast(L3h1[:, :]), _mmcast(h1sb[:, :]),
                        start=False, stop=False,
                    )
                    nc.tensor.matmul(
                        h3p[:, :], _mmcast(L3x[:, :]), xi, start=False, stop=True
                    )

                    rsl = slice(s * (SUB // K), (s + 1) * (SUB // K))
                    nc.vector.reduce_max(
                        h1max[:, rsl],
                        h1sb[:, :].rearrange("p (r k) -> p r k", k=K),
                        axis=mybir.AxisListType.X,
                    )
                    nc.vector.reduce_max(
                        h2max[:, rsl],
                        h2sb[:, :].rearrange("p (r k) -> p r k", k=K),
                        axis=mybir.AxisListType.X,
                    )
                    nc.vector.reduce_max(
                        h3max[:, rsl],
                        h3p[:, :].rearrange("p (r k) -> p r k", k=K),
                        axis=mybir.AxisListType.X,
                    )

                nc.sync.dma_start(o[0:C, c0 : c0 + 128], h3max[:, :])
                nc.sync.dma_start(o[C : 2 * C, c0 : c0 + 128], h2max[:, :])
                nc.sync.dma_start(o[2 * C : 3 * C, c0 : c0 + 128], h1max[:, :])

    return nc


def make_core_inputs(x, pos, W1, b1, W2, b2, W3, b3):
    """Host-side shard prep: per-core rotated transposes (layout only)."""
    ndiag = np.zeros((128, 128), np.float32)
    np.fill_diagonal(ndiag, NEG_BIG)
    ins = []
    for core in range(NCORES):
        b, half = divmod(core, 2)
        r0 = half * Q
        perm = (np.arange(N) + r0) % N
        ins.append(
            {
                "xT": np.ascontiguousarray(x[b][perm].T),
                "posT": np.ascontiguousarray(pos[b][perm].T),
                "W1": np.ascontiguousarray(W1),
                "W2": np.ascontiguousarray(W2),
                "W3": np.ascontiguousarray(W3),
                "b1": np.ascontiguousarray(b1.reshape(1, C)),
                "b2": np.ascontiguousarray(b2.reshape(1, C)),
                "b3": np.ascontiguousarray(b3.reshape(1, C)),
                "ndiag": ndiag,
            }
        )
    return ins


def assemble_output(results):
    out = np.empty((B, N, 3 * C + D), np.float32)
    for core in range(NCORES):
        b, half = divmod(core, 2)
        r0 = half * Q
        out[b, r0 : r0 + Q, :] = results[core]["o"].T
    return out


def kernel(x, pos, W1, b1, W2, b2, W3, b3):
    from concourse.bass_utils import run_bass_kernel_spmd

    ins = make_core_inputs(
        np.asarray(x), np.asarray(pos), np.asarray(W1), np.asarray(b1),
        np.asarray(W2), np.asarray(b2), np.asarray(W3), np.asarray(b3),
    )
    nc = build_nc()
    res = run_bass_kernel_spmd(nc, ins, core_ids=list(range(NCORES)))
    return assemble_output(res.results)
